# revision 47
# baseline (speedup 1.0000x reference)
import sys

sys.path.insert(0, "/opt/trn_rl_repo")

import numpy as np
import ml_dtypes

from concourse import bass, tile, bacc
from concourse.bass_utils import run_bass_kernel_spmd

WORLD, M, N, K_LOCAL = 8, 8192, 2048, 256
M_PER = M // WORLD  # 1024
KT = WORLD * K_LOCAL  # 2048 total contraction dim
NK = KT // 128  # 16 k-tiles
NCH = 512
NNC = N // NCH  # 4 n-chunks
NM = M_PER // 128  # 8 m-tiles
F32 = bass.mybir.dt.float32
BF16 = bass.mybir.dt.bfloat16

# k-tile groups per batched input DMA: fine-grained up front so the first
# matmul chains can start, coarse later to keep DMA-issue count low.
A_GROUPS = [(4, 6), (6, 8), (8, 10), (10, 12), (12, 14), (14, 16)]
W_GROUPS = [(1, 2), (2, 4), (4, 8), (8, 12), (12, 16)]

_NC = None


def _build():
    # Sharding: the leading "rank" axis of A/weight is just a K-shard index,
    # so instead of K-parallel + reduce-scatter, shard over M: core r does the
    # full K=2048 reduction for its own [1024, 2048] output block. No
    # collective, no 64MiB partial staging.
    nc = bacc.Bacc(None, target_bir_lowering=False, num_devices=WORLD)
    At = nc.dram_tensor("a_t", [KT, M_PER], BF16, kind="ExternalInput")
    Wt = nc.dram_tensor("w_t", [NNC * KT, NCH], BF16, kind="ExternalInput")
    out = nc.dram_tensor("out", [M_PER, N], BF16, kind="ExternalOutput")

    with tile.TileContext(nc) as tc:
        with (
            tc.tile_pool(name="ab", bufs=1) as ab,
            tc.tile_pool(name="wb", bufs=1) as wb,
            tc.tile_pool(name="ob", bufs=4) as ob,
            tc.tile_pool(name="ps", bufs=8, space="PSUM") as ps,
        ):
            A_sb = ab.tile([128, NK, M_PER], BF16)  # 32 KB/partition
            W_sb = wb.tile([128, NNC, NK, NCH], BF16)  # 64 KB/partition

            # Batched loads in k-pairs: fewer issue slots (~0.65us each on the
            # sync engine) gets later k-tiles requested sooner. W0 k0-1 first
            # (the first chains' biggest input), then the m0-column sliver and
            # m1-7 remainder for k0-1, then the same trio for k2-3.
            for kp in range(2):
                k0, k1 = kp * 2, kp * 2 + 2
                nc.sync.dma_start(
                    W_sb[:, 0, k0:k1, :],
                    Wt[k0 * 128 : k1 * 128, :].rearrange(
                        "(t p) n -> p t n", p=128
                    ),
                )
                nc.sync.dma_start(
                    A_sb[:, k0:k1, 0:128],
                    At[k0 * 128 : k1 * 128, 0:128].rearrange(
                        "(t p) m -> p t m", p=128
                    ),
                )
                # remainder split by m-group: chains m1-3 unstall on the
                # smaller first piece instead of the whole 448KB transfer
                nc.sync.dma_start(
                    A_sb[:, k0:k1, 128:512],
                    At[k0 * 128 : k1 * 128, 128:512].rearrange(
                        "(t p) m -> p t m", p=128
                    ),
                )
                nc.sync.dma_start(
                    A_sb[:, k0:k1, 512:M_PER],
                    At[k0 * 128 : k1 * 128, 512:M_PER].rearrange(
                        "(t p) m -> p t m", p=128
                    ),
                )
            wg = iter(W_GROUPS[2:])
            for k0, k1 in A_GROUPS:
                nc.sync.dma_start(
                    A_sb[:, k0:k1, :],
                    At[k0 * 128 : k1 * 128, :].rearrange(
                        "(t p) m -> p t m", p=128
                    ),
                )
                w = next(wg, None)
                if w is not None:
                    nc.sync.dma_start(
                        W_sb[:, 0, w[0] : w[1], :],
                        Wt[w[0] * 128 : w[1] * 128, :].rearrange(
                            "(t p) n -> p t n", p=128
                        ),
                    )
            # W chunks 1-3 aren't consumed until ~40/70/100us in, but the DMA
            # rings would otherwise start them immediately and steal HBM
            # bandwidth from the A/W0 k-tiles the first chains are waiting on.
            # Each 4-ktile group gets a 1-element sliver copy (WAR dep) on the
            # A tail, so they all hold until A has landed, then burst in
            # parallel at full bandwidth — and the consumer only waits on the
            # 512KB group it needs, not a whole 2MiB chunk.
            tail = A_sb[:, NK - 1, 0:1]
            for nci in range(1, NNC):
                for g0 in range(0, NK, 4):
                    nc.gpsimd.tensor_copy(W_sb[:, nci, g0, 0:1], tail)
                    nc.sync.dma_start(
                        W_sb[:, nci, g0 : g0 + 4, :],
                        Wt[nci * KT + g0 * 128 : nci * KT + (g0 + 4) * 128, :].rearrange(
                            "(t p) n -> p t n", p=128
                        ),
                    )

            for nci in range(NNC):
                for mi in range(NM):
                    if nci == NNC - 1 and mi == NM - 1:
                        # Split the very last chain into two 256-col halves so
                        # the first half's copy+DMA overlap the second half's
                        # matmuls — shorter post-stream tail.
                        for h in range(2):
                            acc = ps.tile([128, 256], F32, tag="acc", name="acch")
                            for ki in range(NK):
                                nc.tensor.matmul(
                                    acc[:],
                                    A_sb[:, ki, mi * 128 : (mi + 1) * 128],
                                    W_sb[:, nci, ki, h * 256 : (h + 1) * 256],
                                    start=(ki == 0),
                                    stop=(ki == NK - 1),
                                )
                            row = ob.tile([128, 256], BF16, tag="row", name="rowh")
                            nc.vector.tensor_copy(row[:], acc[:])
                            # sync queue is idle by now — these last two issues
                            # overlap the scalar queue's preceding out-DMAs
                            nc.sync.dma_start(
                                out[
                                    mi * 128 : (mi + 1) * 128,
                                    nci * NCH + h * 256 : nci * NCH + (h + 1) * 256,
                                ],
                                row[:],
                            )
                        continue
                    acc = ps.tile([128, NCH], F32, tag="acc")
                    for ki in range(NK):
                        nc.tensor.matmul(
                            acc[:],
                            A_sb[:, ki, mi * 128 : (mi + 1) * 128],
                            W_sb[:, nci, ki, :],
                            start=(ki == 0),
                            stop=(ki == NK - 1),
                        )
                    row = ob.tile([128, NCH], BF16, tag="row")
                    nc.vector.tensor_copy(row[:], acc[:])
                    # outputs go out on the scalar engine's DMA queue so they
                    # never head-of-line-block input loads on the sync queue
                    nc.scalar.dma_start(
                        out[mi * 128 : (mi + 1) * 128, nci * NCH : (nci + 1) * NCH],
                        row[:],
                    )
    nc.compile()
    return nc


def _prep(A, weight):
    A = np.asarray(A, dtype=np.float32)
    weight = np.asarray(weight, dtype=np.float32)
    # weight [world, N, K_local] -> K-major [KT, N], then n-chunk-major
    # [NNC, KT, NCH] flattened so each k-tile slab is contiguous.
    wt = weight.transpose(0, 2, 1).reshape(KT, N)
    wt4 = (
        np.ascontiguousarray(wt.reshape(KT, NNC, NCH).transpose(1, 0, 2))
        .reshape(NNC * KT, NCH)
        .astype(ml_dtypes.bfloat16)
    )
    in_maps = []
    for r in range(WORLD):
        strip = A[:, r * M_PER : (r + 1) * M_PER, :]  # [world, 1024, K_local]
        at = (
            np.ascontiguousarray(strip.transpose(0, 2, 1))
            .reshape(KT, M_PER)
            .astype(ml_dtypes.bfloat16)
        )
        in_maps.append({"a_t": at, "w_t": wt4})
    return in_maps


def kernel(A, weight, _trace=False):
    global _NC
    if _NC is None:
        _NC = _build()
    in_maps = _prep(A, weight)
    res = run_bass_kernel_spmd(
        _NC, in_maps, core_ids=list(range(WORLD)), trace=_trace
    )
    out = np.stack(
        [res.results[r]["out"].astype(np.float32) for r in range(WORLD)], axis=0
    )
    if _trace:
        return out, res
    return out
